# revision 6
# baseline (speedup 1.0000x reference)
"""ConvolutionalMultiheadAttention on 8 TRN2 NeuronCores (Bass/Tile).

Problem: S=1024, B=8, D=256, H=8 heads (head_dim=256), conv kernel K=3.
  q = causal_conv(Q, Wq, bq); k = causal_conv(K, Wk, bk); v = V @ Wv.T + bv
  attn = softmax(q k^T / 16 + mask); ctx = attn @ v; out = ctx @ Wo.T + bo
Returns (out [S,B,D], attn [B,H,S,S]) like the reference.

Sharding: data-parallel over batch — core b handles batch element b.
Zero collectives; outputs are disjoint slices.

Per-core dataflow (all matmuls in float32r — full-rate fp32 storage,
~1e-4 relative precision):
  - Host pre-transposes inputs/weights so every matmul contraction dim
    is on SBUF partitions, and the causal conv becomes 3 shifted matmuls.
  - qT/kT [d, s] per head via conv matmuls (bias+scale fused in the ACT
    PSUM drain; q pre-scaled by 1/16).
  - scores are computed in BOTH orientations on the PE (cheaper than
    transposing attn): scoresT [t, s] feeds AV; natural [s, t] feeds the
    attn output.
  - softmax without max-subtraction (scores are O(1) here; exp is exact
    to 2 ulp): ACT Exp drains PSUM directly, accum_out gives row sums.
  - AV uses unnormalized exp; the 1/rowsum is applied per head at the
    out-projection drain (per-partition scalar on DVE).
  - v bias and output bias fold into a constant added on the host:
    attn rows sum to 1, so their contribution is bo + Wo @ bv.
"""
import os
from contextlib import ExitStack

import numpy as np

S, B, D, H, KW = 1024, 8, 256, 8, 3
N_CORES = 8
SCALE = 1.0 / 16.0

_BUILD_CACHE = {}


def _build(with_mask: bool):
    import concourse.bacc as bacc
    import concourse.tile as tile
    import concourse.mybir as mybir

    f32 = mybir.dt.float32
    f32r = mybir.dt.float32r
    Ident = mybir.ActivationFunctionType.Identity
    Exp = mybir.ActivationFunctionType.Exp
    MULT = mybir.AluOpType.mult
    ADD = mybir.AluOpType.add

    nc = bacc.Bacc(
        "TRN2", target_bir_lowering=False, debug=False, num_devices=N_CORES
    )

    # xq/xk are zero-padded on the host with KW-1 leading columns so the
    # causal conv's shifted reads never go out of range.
    SP = S + KW - 1
    xq_d = nc.dram_tensor("xq", [2, 128, SP], f32r, kind="ExternalInput").ap()
    xk_d = nc.dram_tensor("xk", [2, 128, SP], f32r, kind="ExternalInput").ap()
    xv_d = nc.dram_tensor("xv", [2, 128, S], f32r, kind="ExternalInput").ap()
    wq_d = nc.dram_tensor("wq", [2, 128, KW, H * D], f32r, kind="ExternalInput").ap()
    wk_d = nc.dram_tensor("wk", [2, 128, KW, H * D], f32r, kind="ExternalInput").ap()
    wv_d = nc.dram_tensor("wv", [2, 128, H * D], f32r, kind="ExternalInput").ap()
    wo_d = nc.dram_tensor("wo", [16, 128, D], f32r, kind="ExternalInput").ap()
    bq_d = nc.dram_tensor("bq16", [128, 16], f32, kind="ExternalInput").ap()
    bk_d = nc.dram_tensor("bkb", [128, 16], f32, kind="ExternalInput").ap()
    if with_mask:
        mask_d = nc.dram_tensor("mask", [S, S], f32, kind="ExternalInput").ap()
        maskT_d = nc.dram_tensor("maskT", [S, S], f32, kind="ExternalInput").ap()
    attn_d = nc.dram_tensor("attn_o", [H, S, S], f32, kind="ExternalOutput").ap()
    out_d = nc.dram_tensor("out_o", [S, D], f32, kind="ExternalOutput").ap()

    def r(ap):
        return ap

    with tile.TileContext(nc) as tc, ExitStack() as ctx:
        xp = ctx.enter_context(tc.tile_pool(name="xp", bufs=1))
        bp = ctx.enter_context(tc.tile_pool(name="bp", bufs=1))
        wp = ctx.enter_context(tc.tile_pool(name="wp", bufs=2))
        qk = ctx.enter_context(tc.tile_pool(name="qk", bufs=1))
        vpool = ctx.enter_context(tc.tile_pool(name="vpool", bufs=1))
        ep = ctx.enter_context(tc.tile_pool(name="ep", bufs=1))
        stp = ctx.enter_context(tc.tile_pool(name="stp", bufs=3))
        cp = ctx.enter_context(tc.tile_pool(name="cp", bufs=1))
        oa = ctx.enter_context(tc.tile_pool(name="oa", bufs=1))
        smp = ctx.enter_context(tc.tile_pool(name="smp", bufs=16))
        if with_mask:
            mp = ctx.enter_context(tc.tile_pool(name="mp", bufs=3))
        ps_big = ctx.enter_context(tc.tile_pool(name="ps_big", bufs=2, space="PSUM"))
        ps_sm = ctx.enter_context(tc.tile_pool(name="ps_sm", bufs=2, space="PSUM"))
        ps_tn = ctx.enter_context(tc.tile_pool(name="ps_tn", bufs=2, space="PSUM"))

        xq_t = xp.tile([128, 2, SP], f32r)
        xk_t = xp.tile([128, 2, SP], f32r)
        xv_t = xp.tile([128, 2, S], f32r)
        for cb in range(2):
            nc.sync.dma_start(xq_t[:, cb, :], xq_d[cb])
            nc.sync.dma_start(xk_t[:, cb, :], xk_d[cb])
            nc.sync.dma_start(xv_t[:, cb, :], xv_d[cb])
        bq_t = bp.tile([128, 16], f32)
        bk_t = bp.tile([128, 16], f32)
        nc.sync.dma_start(bq_t[:], bq_d[:])
        nc.sync.dma_start(bk_t[:], bk_d[:])

        out_acc = oa.tile([128, 8, D], f32)

        for h in range(H):
            ho = h * D
            wq_t = wp.tile([128, 2, KW, D], f32r, tag="wq")
            wk_t = wp.tile([128, 2, KW, D], f32r, tag="wk")
            wv_t = wp.tile([128, 2, D], f32r, tag="wv")
            wo_t = wp.tile([128, 2, D], f32r, tag="wo")
            for cb in range(2):
                nc.sync.dma_start(wq_t[:, cb], wq_d[cb, :, :, ho:ho + D])
                nc.sync.dma_start(wk_t[:, cb], wk_d[cb, :, :, ho:ho + D])
                nc.sync.dma_start(wv_t[:, cb, :], wv_d[cb, :, ho:ho + D])
                nc.sync.dma_start(wo_t[:, cb, :], wo_d[2 * h + cb])

            # --- q/k projections (causal conv as 3 shifted matmuls) ---
            # qT[d, s] = sum_{cb,tap} WqT[c, tap, d] * x_pad[c, s + tap]
            # (x_pad has KW-1 leading zero columns, so index s+tap is the
            # original x[s - (KW-1) + tap] with causal zero fill).
            qT = qk.tile([128, 2, S], f32r, tag="qT")
            kT = qk.tile([128, 2, S], f32r, tag="kT")
            for (w_t, x_t, o_t, bias_t, scl) in (
                (wq_t, xq_t, qT, bq_t, SCALE),
                (wk_t, xk_t, kT, bk_t, 1.0),
            ):
                for db in range(2):
                    for sh in range(2):
                        s0 = sh * 512
                        ps = ps_sm.tile([128, 512], f32, tag="ps")
                        for cb in range(2):
                            for tap in range(KW):
                                nc.tensor.matmul(
                                    ps[:],
                                    r(w_t[:, cb, tap, db * 128:(db + 1) * 128]),
                                    r(x_t[:, cb, s0 + tap:s0 + tap + 512]),
                                    start=(cb == 0 and tap == 0),
                                    stop=(cb == 1 and tap == KW - 1),
                                )
                        nc.scalar.activation(
                            o_t[:, db, s0:s0 + 512], ps[:], Ident,
                            bias=bias_t[:, 2 * h + db:2 * h + db + 1],
                            scale=scl,
                        )

            # --- v projection: v[t, d] ---
            v_t = vpool.tile([128, 8, D], f32r)
            for tb in range(8):
                ps = ps_tn.tile([128, D], f32, tag="ps")
                for cb in range(2):
                    nc.tensor.matmul(
                        ps[:],
                        r(xv_t[:, cb, tb * 128:(tb + 1) * 128]),
                        r(wv_t[:, cb, :]),
                        start=(cb == 0), stop=(cb == 1),
                    )
                nc.vector.tensor_copy(v_t[:, tb, :], ps[:])

            # --- scoresT [t, s] -> exp -> expT (feeds AV) ---
            expT = ep.tile([128, 8, S], f32r)
            for tb in range(8):
                ps = ps_big.tile([128, S], f32, tag="ps")
                for sh in range(2):
                    for db in range(2):
                        nc.tensor.matmul(
                            ps[:, sh * 512:(sh + 1) * 512],
                            r(kT[:, db, tb * 128:(tb + 1) * 128]),
                            r(qT[:, db, sh * 512:(sh + 1) * 512]),
                            start=(db == 0), stop=(db == 1),
                        )
                if with_mask:
                    m_t = mp.tile([128, S], f32, tag="mT")
                    nc.sync.dma_start(
                        m_t[:], maskT_d[tb * 128:(tb + 1) * 128, :]
                    )
                    nc.vector.tensor_add(ps[:], ps[:], m_t[:])
                nc.scalar.activation(expT[:, tb, :], ps[:], Exp)

            # --- natural scores [s, t] -> softmax -> attn out ---
            recips = []
            for sb in range(8):
                ps = ps_big.tile([128, S], f32, tag="ps")
                for th in range(2):
                    for db in range(2):
                        nc.tensor.matmul(
                            ps[:, th * 512:(th + 1) * 512],
                            r(qT[:, db, sb * 128:(sb + 1) * 128]),
                            r(kT[:, db, th * 512:(th + 1) * 512]),
                            start=(db == 0), stop=(db == 1),
                        )
                if with_mask:
                    m_t = mp.tile([128, S], f32, tag="mN")
                    nc.sync.dma_start(m_t[:], mask_d[sb * 128:(sb + 1) * 128, :])
                    nc.vector.tensor_add(ps[:], ps[:], m_t[:])
                ex = stp.tile([128, S], f32)
                ssum = smp.tile([128, 1], f32, tag="sum")
                nc.scalar.activation(ex[:], ps[:], Exp, accum_out=ssum[:])
                rec = smp.tile([128, 1], f32, tag="rec")
                nc.vector.reciprocal(rec[:], ssum[:])
                nc.vector.tensor_scalar_mul(ex[:], ex[:], rec[:])
                nc.sync.dma_start(attn_d[h, sb * 128:(sb + 1) * 128, :], ex[:])
                recips.append(rec)

            # --- AV: ctxT[d, s] = sum_t v[t, d] * expT[t, s] ---
            ctxT = cp.tile([128, 2, S], f32r)
            for db in range(2):
                ps = ps_big.tile([128, S], f32, tag="ps")
                for sh in range(2):
                    for tb in range(8):
                        nc.tensor.matmul(
                            ps[:, sh * 512:(sh + 1) * 512],
                            r(v_t[:, tb, db * 128:(db + 1) * 128]),
                            r(expT[:, tb, sh * 512:(sh + 1) * 512]),
                            start=(tb == 0), stop=(tb == 7),
                        )
                nc.vector.tensor_copy(ctxT[:, db, :], ps[:])

            # --- out projection, normalized per head, accumulated ---
            for sb in range(8):
                ps = ps_tn.tile([128, D], f32, tag="ps")
                for db in range(2):
                    nc.tensor.matmul(
                        ps[:],
                        r(ctxT[:, db, sb * 128:(sb + 1) * 128]),
                        r(wo_t[:, db, :]),
                        start=(db == 0), stop=(db == 1),
                    )
                if h == 0:
                    nc.vector.tensor_scalar_mul(
                        out_acc[:, sb, :], ps[:], recips[sb][:]
                    )
                else:
                    nc.vector.scalar_tensor_tensor(
                        out_acc[:, sb, :], ps[:], recips[sb][:],
                        out_acc[:, sb, :], MULT, ADD,
                    )

        nc.sync.dma_start(
            out_d.rearrange("(sb p) d -> p sb d", p=128), out_acc[:]
        )

    nc.compile()
    return nc


def _get_nc(with_mask: bool):
    if with_mask not in _BUILD_CACHE:
        _BUILD_CACHE[with_mask] = _build(with_mask)
    return _BUILD_CACHE[with_mask]


def prepare_in_maps(inputs):
    """Host-side re-layout. Returns (in_maps, out_const) where
    out_const [D] = bo + Wo @ bv must be added to every out row."""
    Q = np.asarray(inputs["Q"], np.float32)
    K = np.asarray(inputs["K"], np.float32)
    V = np.asarray(inputs["V"], np.float32)
    mask = np.asarray(inputs["attn_mask"], np.float32)
    Wq = np.asarray(inputs["Wq"], np.float32)
    bq = np.asarray(inputs["bq"], np.float32)
    Wk = np.asarray(inputs["Wk"], np.float32)
    bk = np.asarray(inputs["bk"], np.float32)
    Wv = np.asarray(inputs["Wv"], np.float32)
    bv = np.asarray(inputs["bv"], np.float32)
    Wo = np.asarray(inputs["Wo"], np.float32)
    bo = np.asarray(inputs["bo"], np.float32)

    with_mask = bool(np.any(mask))
    # [o, c, t] -> [c, t, o] -> [2, 128, KW, H*D]
    wq_h = np.ascontiguousarray(
        Wq.transpose(1, 2, 0).reshape(2, 128, KW, H * D))
    wk_h = np.ascontiguousarray(
        Wk.transpose(1, 2, 0).reshape(2, 128, KW, H * D))
    wv_h = np.ascontiguousarray(Wv.T.reshape(2, 128, H * D))
    wo_h = np.ascontiguousarray(Wo.T.reshape(16, 128, D))
    bq_h = np.ascontiguousarray((bq * SCALE).reshape(16, 128).T)
    bk_h = np.ascontiguousarray(bk.reshape(16, 128).T)
    out_const = (bo + Wo @ bv).astype(np.float32)

    def pad_x(x2d):  # [S, D] -> [2, 128, S + KW - 1] with leading zeros
        xt = np.zeros((D, S + KW - 1), np.float32)
        xt[:, KW - 1:] = x2d.T
        return np.ascontiguousarray(xt.reshape(2, 128, S + KW - 1))

    in_maps = []
    for b in range(N_CORES):
        m = {
            "xq": pad_x(Q[:, b, :]),
            "xk": pad_x(K[:, b, :]),
            "xv": np.ascontiguousarray(V[:, b, :].T.reshape(2, 128, S)),
            "wq": wq_h, "wk": wk_h, "wv": wv_h, "wo": wo_h,
            "bq16": bq_h, "bkb": bk_h,
        }
        if with_mask:
            m["mask"] = mask
            m["maskT"] = np.ascontiguousarray(mask.T)
        in_maps.append(m)
    return in_maps, out_const, with_mask


def postprocess(results, out_const):
    out = np.stack([results[b]["out_o"] for b in range(N_CORES)], axis=1)
    out = out + out_const[None, None, :]
    attn = np.stack([results[b]["attn_o"] for b in range(N_CORES)], axis=0)
    return out.astype(np.float32), attn


def kernel(**inputs):
    from concourse.bass_utils import run_bass_kernel_spmd

    in_maps, out_const, with_mask = prepare_in_maps(inputs)
    nc = _get_nc(with_mask)
    res = run_bass_kernel_spmd(nc, in_maps, list(range(N_CORES)))
    return postprocess(res.results, out_const)


# revision 7
# speedup vs baseline: 4.5822x; 4.5822x over previous
"""ConvolutionalMultiheadAttention on 8 TRN2 NeuronCores (Bass/Tile).

Problem: S=1024, B=8, D=256, H=8 heads (head_dim=256), conv kernel K=3.
  q = causal_conv(Q, Wq, bq); k = causal_conv(K, Wk, bk); v = V @ Wv.T + bv
  attn = softmax(q k^T / 16 + mask); ctx = attn @ v; out = ctx @ Wo.T + bo
Returns (out [S,B,D], attn [B,H,S,S]) like the reference.

Sharding: data-parallel over batch — core b handles batch element b.
Zero collectives; outputs are disjoint slices.

Per-core dataflow (all matmuls in float32r — full-rate fp32 storage,
~1e-4 relative precision):
  - Host pre-transposes inputs/weights so every matmul contraction dim
    is on SBUF partitions, and the causal conv becomes 3 shifted matmuls.
  - qT/kT [d, s] per head via conv matmuls (bias+scale fused in the ACT
    PSUM drain; q pre-scaled by 1/16).
  - scores are computed in BOTH orientations on the PE (cheaper than
    transposing attn): scoresT [t, s] feeds AV; natural [s, t] feeds the
    attn output.
  - softmax without max-subtraction (scores are O(1) here; exp is exact
    to 2 ulp): ACT Exp drains PSUM directly, accum_out gives row sums.
  - AV uses unnormalized exp; the 1/rowsum is applied per head at the
    out-projection drain (per-partition scalar on DVE).
  - v bias and output bias fold into a constant added on the host:
    attn rows sum to 1, so their contribution is bo + Wo @ bv.
"""
import os
from contextlib import ExitStack

import numpy as np

S, B, D, H, KW = 1024, 8, 256, 8, 3
N_CORES = 8
SCALE = 1.0 / 16.0

_BUILD_CACHE = {}
MM_DT = os.environ.get("KERNEL_MM_DT", "bfloat16")


def _build(with_mask: bool, mm_dt_name: str = None):
    mm_dt_name = mm_dt_name or MM_DT
    import concourse.bacc as bacc
    import concourse.tile as tile
    import concourse.mybir as mybir

    f32 = mybir.dt.float32
    f32r = getattr(mybir.dt, mm_dt_name)
    Ident = mybir.ActivationFunctionType.Identity
    Exp = mybir.ActivationFunctionType.Exp
    MULT = mybir.AluOpType.mult
    ADD = mybir.AluOpType.add

    nc = bacc.Bacc(
        "TRN2", target_bir_lowering=False, debug=False, num_devices=N_CORES
    )

    # xq/xk are zero-padded on the host with KW-1 leading columns so the
    # causal conv's shifted reads never go out of range.
    SP = S + KW - 1
    xq_d = nc.dram_tensor("xq", [2, 128, SP], f32r, kind="ExternalInput").ap()
    xk_d = nc.dram_tensor("xk", [2, 128, SP], f32r, kind="ExternalInput").ap()
    xv_d = nc.dram_tensor("xv", [2, 128, S], f32r, kind="ExternalInput").ap()
    wq_d = nc.dram_tensor("wq", [2, 128, KW, H * D], f32r, kind="ExternalInput").ap()
    wk_d = nc.dram_tensor("wk", [2, 128, KW, H * D], f32r, kind="ExternalInput").ap()
    wv_d = nc.dram_tensor("wv", [2, 128, H * D], f32r, kind="ExternalInput").ap()
    wo_d = nc.dram_tensor("wo", [16, 128, D], f32r, kind="ExternalInput").ap()
    bq_d = nc.dram_tensor("bq16", [128, 16], f32, kind="ExternalInput").ap()
    bk_d = nc.dram_tensor("bkb", [128, 16], f32, kind="ExternalInput").ap()
    if with_mask:
        mask_d = nc.dram_tensor("mask", [S, S], f32, kind="ExternalInput").ap()
        maskT_d = nc.dram_tensor("maskT", [S, S], f32, kind="ExternalInput").ap()
    attn_d = nc.dram_tensor("attn_o", [H, S, S], f32, kind="ExternalOutput").ap()
    out_d = nc.dram_tensor("out_o", [S, D], f32, kind="ExternalOutput").ap()

    def r(ap):
        return ap

    with tile.TileContext(nc) as tc, ExitStack() as ctx:
        xp = ctx.enter_context(tc.tile_pool(name="xp", bufs=1))
        bp = ctx.enter_context(tc.tile_pool(name="bp", bufs=1))
        wp = ctx.enter_context(tc.tile_pool(name="wp", bufs=2))
        qk = ctx.enter_context(tc.tile_pool(name="qk", bufs=1))
        vpool = ctx.enter_context(tc.tile_pool(name="vpool", bufs=1))
        ep = ctx.enter_context(tc.tile_pool(name="ep", bufs=1))
        stp = ctx.enter_context(tc.tile_pool(name="stp", bufs=3))
        cp = ctx.enter_context(tc.tile_pool(name="cp", bufs=1))
        oa = ctx.enter_context(tc.tile_pool(name="oa", bufs=1))
        smp = ctx.enter_context(tc.tile_pool(name="smp", bufs=16))
        if with_mask:
            mp = ctx.enter_context(tc.tile_pool(name="mp", bufs=3))
        ps_big = ctx.enter_context(tc.tile_pool(name="ps_big", bufs=2, space="PSUM"))
        ps_sm = ctx.enter_context(tc.tile_pool(name="ps_sm", bufs=2, space="PSUM"))
        ps_tn = ctx.enter_context(tc.tile_pool(name="ps_tn", bufs=2, space="PSUM"))

        xq_t = xp.tile([128, 2, SP], f32r)
        xk_t = xp.tile([128, 2, SP], f32r)
        xv_t = xp.tile([128, 2, S], f32r)
        for cb in range(2):
            nc.sync.dma_start(xq_t[:, cb, :], xq_d[cb])
            nc.sync.dma_start(xk_t[:, cb, :], xk_d[cb])
            nc.sync.dma_start(xv_t[:, cb, :], xv_d[cb])
        bq_t = bp.tile([128, 16], f32)
        bk_t = bp.tile([128, 16], f32)
        nc.sync.dma_start(bq_t[:], bq_d[:])
        nc.sync.dma_start(bk_t[:], bk_d[:])

        out_acc = oa.tile([128, 8, D], f32)

        for h in range(H):
            ho = h * D
            wq_t = wp.tile([128, 2, KW, D], f32r, tag="wq")
            wk_t = wp.tile([128, 2, KW, D], f32r, tag="wk")
            wv_t = wp.tile([128, 2, D], f32r, tag="wv")
            wo_t = wp.tile([128, 2, D], f32r, tag="wo")
            for cb in range(2):
                nc.sync.dma_start(wq_t[:, cb], wq_d[cb, :, :, ho:ho + D])
                nc.sync.dma_start(wk_t[:, cb], wk_d[cb, :, :, ho:ho + D])
                nc.sync.dma_start(wv_t[:, cb, :], wv_d[cb, :, ho:ho + D])
                nc.sync.dma_start(wo_t[:, cb, :], wo_d[2 * h + cb])

            # --- q/k projections (causal conv as 3 shifted matmuls) ---
            # qT[d, s] = sum_{cb,tap} WqT[c, tap, d] * x_pad[c, s + tap]
            # (x_pad has KW-1 leading zero columns, so index s+tap is the
            # original x[s - (KW-1) + tap] with causal zero fill).
            qT = qk.tile([128, 2, S], f32r, tag="qT")
            kT = qk.tile([128, 2, S], f32r, tag="kT")
            for (w_t, x_t, o_t, bias_t, scl) in (
                (wq_t, xq_t, qT, bq_t, SCALE),
                (wk_t, xk_t, kT, bk_t, 1.0),
            ):
                for db in range(2):
                    for sh in range(2):
                        s0 = sh * 512
                        ps = ps_sm.tile([128, 512], f32, tag="ps")
                        for cb in range(2):
                            for tap in range(KW):
                                nc.tensor.matmul(
                                    ps[:],
                                    r(w_t[:, cb, tap, db * 128:(db + 1) * 128]),
                                    r(x_t[:, cb, s0 + tap:s0 + tap + 512]),
                                    start=(cb == 0 and tap == 0),
                                    stop=(cb == 1 and tap == KW - 1),
                                )
                        nc.scalar.activation(
                            o_t[:, db, s0:s0 + 512], ps[:], Ident,
                            bias=bias_t[:, 2 * h + db:2 * h + db + 1],
                            scale=scl,
                        )

            # --- v projection: v[t, d] ---
            v_t = vpool.tile([128, 8, D], f32r)
            for tb in range(8):
                ps = ps_tn.tile([128, D], f32, tag="ps")
                for cb in range(2):
                    nc.tensor.matmul(
                        ps[:],
                        r(xv_t[:, cb, tb * 128:(tb + 1) * 128]),
                        r(wv_t[:, cb, :]),
                        start=(cb == 0), stop=(cb == 1),
                    )
                nc.vector.tensor_copy(v_t[:, tb, :], ps[:])

            # --- scoresT [t, s] -> exp -> expT (feeds AV) ---
            expT = ep.tile([128, 8, S], f32r)
            for tb in range(8):
                ps = ps_big.tile([128, S], f32, tag="ps")
                for sh in range(2):
                    for db in range(2):
                        nc.tensor.matmul(
                            ps[:, sh * 512:(sh + 1) * 512],
                            r(kT[:, db, tb * 128:(tb + 1) * 128]),
                            r(qT[:, db, sh * 512:(sh + 1) * 512]),
                            start=(db == 0), stop=(db == 1),
                        )
                if with_mask:
                    m_t = mp.tile([128, S], f32, tag="mT")
                    nc.sync.dma_start(
                        m_t[:], maskT_d[tb * 128:(tb + 1) * 128, :]
                    )
                    nc.vector.tensor_add(ps[:], ps[:], m_t[:])
                nc.scalar.activation(expT[:, tb, :], ps[:], Exp)

            # --- natural scores [s, t] -> softmax -> attn out ---
            recips = []
            for sb in range(8):
                ps = ps_big.tile([128, S], f32, tag="ps")
                for th in range(2):
                    for db in range(2):
                        nc.tensor.matmul(
                            ps[:, th * 512:(th + 1) * 512],
                            r(qT[:, db, sb * 128:(sb + 1) * 128]),
                            r(kT[:, db, th * 512:(th + 1) * 512]),
                            start=(db == 0), stop=(db == 1),
                        )
                if with_mask:
                    m_t = mp.tile([128, S], f32, tag="mN")
                    nc.sync.dma_start(m_t[:], mask_d[sb * 128:(sb + 1) * 128, :])
                    nc.vector.tensor_add(ps[:], ps[:], m_t[:])
                ex = stp.tile([128, S], f32)
                ssum = smp.tile([128, 1], f32, tag="sum")
                nc.scalar.activation(ex[:], ps[:], Exp, accum_out=ssum[:])
                rec = smp.tile([128, 1], f32, tag="rec")
                nc.vector.reciprocal(rec[:], ssum[:])
                nc.vector.tensor_scalar_mul(ex[:], ex[:], rec[:])
                nc.sync.dma_start(attn_d[h, sb * 128:(sb + 1) * 128, :], ex[:])
                recips.append(rec)

            # --- AV: ctxT[d, s] = sum_t v[t, d] * expT[t, s] ---
            ctxT = cp.tile([128, 2, S], f32r)
            for db in range(2):
                ps = ps_big.tile([128, S], f32, tag="ps")
                for sh in range(2):
                    for tb in range(8):
                        nc.tensor.matmul(
                            ps[:, sh * 512:(sh + 1) * 512],
                            r(v_t[:, tb, db * 128:(db + 1) * 128]),
                            r(expT[:, tb, sh * 512:(sh + 1) * 512]),
                            start=(tb == 0), stop=(tb == 7),
                        )
                nc.vector.tensor_copy(ctxT[:, db, :], ps[:])

            # --- out projection, normalized per head, accumulated ---
            for sb in range(8):
                ps = ps_tn.tile([128, D], f32, tag="ps")
                for db in range(2):
                    nc.tensor.matmul(
                        ps[:],
                        r(ctxT[:, db, sb * 128:(sb + 1) * 128]),
                        r(wo_t[:, db, :]),
                        start=(db == 0), stop=(db == 1),
                    )
                if h == 0:
                    nc.vector.tensor_scalar_mul(
                        out_acc[:, sb, :], ps[:], recips[sb][:]
                    )
                else:
                    nc.vector.scalar_tensor_tensor(
                        out_acc[:, sb, :], ps[:], recips[sb][:],
                        out_acc[:, sb, :], MULT, ADD,
                    )

        nc.sync.dma_start(
            out_d.rearrange("(sb p) d -> p sb d", p=128), out_acc[:]
        )

    nc.compile()
    return nc


def _get_nc(with_mask: bool):
    key = (with_mask, MM_DT)
    if key not in _BUILD_CACHE:
        _BUILD_CACHE[key] = _build(with_mask)
    return _BUILD_CACHE[key]


def prepare_in_maps(inputs):
    """Host-side re-layout. Returns (in_maps, out_const) where
    out_const [D] = bo + Wo @ bv must be added to every out row."""
    Q = np.asarray(inputs["Q"], np.float32)
    K = np.asarray(inputs["K"], np.float32)
    V = np.asarray(inputs["V"], np.float32)
    mask = np.asarray(inputs["attn_mask"], np.float32)
    Wq = np.asarray(inputs["Wq"], np.float32)
    bq = np.asarray(inputs["bq"], np.float32)
    Wk = np.asarray(inputs["Wk"], np.float32)
    bk = np.asarray(inputs["bk"], np.float32)
    Wv = np.asarray(inputs["Wv"], np.float32)
    bv = np.asarray(inputs["bv"], np.float32)
    Wo = np.asarray(inputs["Wo"], np.float32)
    bo = np.asarray(inputs["bo"], np.float32)

    with_mask = bool(np.any(mask))
    # [o, c, t] -> [c, t, o] -> [2, 128, KW, H*D]
    wq_h = np.ascontiguousarray(
        Wq.transpose(1, 2, 0).reshape(2, 128, KW, H * D))
    wk_h = np.ascontiguousarray(
        Wk.transpose(1, 2, 0).reshape(2, 128, KW, H * D))
    wv_h = np.ascontiguousarray(Wv.T.reshape(2, 128, H * D))
    wo_h = np.ascontiguousarray(Wo.T.reshape(16, 128, D))
    bq_h = np.ascontiguousarray((bq * SCALE).reshape(16, 128).T)
    bk_h = np.ascontiguousarray(bk.reshape(16, 128).T)
    out_const = (bo + Wo @ bv).astype(np.float32)

    def pad_x(x2d):  # [S, D] -> [2, 128, S + KW - 1] with leading zeros
        xt = np.zeros((D, S + KW - 1), np.float32)
        xt[:, KW - 1:] = x2d.T
        return np.ascontiguousarray(xt.reshape(2, 128, S + KW - 1)).astype(mdt)

    if MM_DT == "float32r":
        mdt = np.float32
    else:
        import ml_dtypes
        mdt = getattr(ml_dtypes, MM_DT)
    wq_h = wq_h.astype(mdt)
    wk_h = wk_h.astype(mdt)
    wv_h = wv_h.astype(mdt)
    wo_h = wo_h.astype(mdt)

    in_maps = []
    for b in range(N_CORES):
        m = {
            "xq": pad_x(Q[:, b, :]),
            "xk": pad_x(K[:, b, :]),
            "xv": np.ascontiguousarray(V[:, b, :].T.reshape(2, 128, S)).astype(mdt),
            "wq": wq_h, "wk": wk_h, "wv": wv_h, "wo": wo_h,
            "bq16": bq_h, "bkb": bk_h,
        }
        if with_mask:
            m["mask"] = mask
            m["maskT"] = np.ascontiguousarray(mask.T)
        in_maps.append(m)
    return in_maps, out_const, with_mask


def postprocess(results, out_const):
    out = np.stack([results[b]["out_o"] for b in range(N_CORES)], axis=1)
    out = out + out_const[None, None, :]
    attn = np.stack([results[b]["attn_o"] for b in range(N_CORES)], axis=0)
    return out.astype(np.float32), attn


def kernel(**inputs):
    from concourse.bass_utils import run_bass_kernel_spmd

    in_maps, out_const, with_mask = prepare_in_maps(inputs)
    nc = _get_nc(with_mask)
    res = run_bass_kernel_spmd(nc, in_maps, list(range(N_CORES)))
    return postprocess(res.results, out_const)
